# revision 4
# baseline (speedup 1.0000x reference)
"""Involution-style per-pixel depthwise 3x3 conv on 8 trn2 NeuronCores.

out[n,c,h,w] = sum_{k=0..8} w[n,c,k,h,w] * x_pad[n,c,h+k//3,w+k%3]  (pad=1)

Sharding: pure data parallel over N=8 -> one sample per core.
Per core: channels C=128 = SBUF partition dim; free dim = H*W pixels.

The kernel is HBM-bandwidth bound: with f32 tensors the mandatory
~52 MB/core at the ~358 GB/s HBM-per-NC limit puts the roofline at
~145 us (measured 142-168 us depending on paired-NC contention). The
2e-2 rel-err gate leaves ~100x of precision headroom, so the device
side runs entirely in FP16: the host converts x and w to fp16 (and
zeroes the border weight columns), the device streams ~26 MB instead
of ~52 MB, and the store is fp16 upcast to f32 on the host. fp16
roofline: ~72-85 us. Measured end-to-end rel err ~4e-4.

Design notes (carried over from the f32 kernel where still relevant):
- x lives in SBUF inside zero guard rows TWICE, at element offsets 98
  (xa) and 99 (xb). A tap (di,dj) reads a fully contiguous window;
  row overruns land in the zero guards (vertical padding); column
  wraps read the neighbor row's edge pixel and are killed by the
  border weight columns zeroed on the host (horizontal padding).
  Why two copies: DVE tensor_tensor only reaches 2x_1P mode (2 fp16
  elems/cycle) when every AP is step-1 AND 4B-aligned. Tap dj=0 reads
  at even element offsets from xa; taps dj=+-1 read at odd offsets of
  the x origin, which are even offsets in xb (x shifted by one). All
  9 products/pixel then run at 2/cycle: ~48 us of DVE, under the DMA
  roofline. xb is built from xa with chunked ScalarE copies (no extra
  HBM traffic).
- Per row-stripe, per row-group (taps sharing a row shift di), one
  DMA brings the 3-tap fp16 weight slab; one DVE tensor_mul per tap
  forms the product in place (fp16 out, exact f32 multiply inside).
- The 9-way tap sum runs on the otherwise-idle TensorE as fp16
  identity matmuls accumulating into f32 PSUM (1 PE-cycle/row).
  ScalarE evacuates PSUM->SBUF with the f32->fp16 cast fused into the
  activation copy; GPSIMD is NOT used for elementwise work (it
  contends with DVE on the shared SBUF port pair).
- The SP HWDGE ring carries ONLY the weight stream; x loads and
  output stores ride the ACT ring so their sem-waits never
  head-of-line block the weight stream. 8 slab buffers keep >2
  stripes of DMA in flight, the first stripe is small so the pipeline
  fills early, and the LAST stripe is 4 rows loaded tap-by-tap with
  the mul/matmul pipelined per tap, so only ~2 us of work remains
  after the final weight byte lands.
"""

import numpy as np

import concourse.bass as bass
import concourse.mybir as mybir
from concourse.bass_utils import run_bass_kernel_spmd
from concourse.masks import make_identity
from concourse.tile import TileContext

N_CORES = 8
C, H, W = 128, 96, 96
HW = H * W
KW = 3

R = 16                # max stripe rows (slab/psum tile sizing)
SL = R * W            # elems per stripe per partition

F16 = mybir.dt.float16
F32 = mybir.dt.float32

# row-groups: (name, first tap k0, row shift di)
GROUPS = (("mid", 3, 0), ("top", 0, -1), ("bot", 6, 1))
# tap order used for PE accumulation: mid first (its slab is DMA'd
# first, so the PE can start earliest), t=1 (dj=0, only needs xa)
# before the xb taps within each group
TAP_ORDER = (
    ("mid", 1), ("mid", 0), ("mid", 2),
    ("top", 1), ("top", 0), ("top", 2),
    ("bot", 1), ("bot", 0), ("bot", 2),
)

# guarded x layouts: xa = [98 zeros | x | 98 zeros], xb = [99 | x | 97]
# Both give 4B-aligned (even-element) window starts: xa serves dj=0
# (offset 98 + row*W, W=96 even), xb serves dj=-1 (98 + row*W) and
# dj=+1 (100 + row*W).
GPA = 98
GPB = 99
GX = GPA + HW + GPA  # 9412, also == GPB + HW + 97

# x arrives in three chunks so the xb shift-copies can chase the DMA
XQ = (21 * W, 37 * W, 38 * W)


def _build() -> bass.Bass:
    # no partition-id parameter: the kernel is SPMD-identical per core
    # and never branches on its core index
    nc = bass.Bass(enable_partition_id=False)
    x_d = nc.dram_tensor("x", [C, HW], F16, kind="ExternalInput")
    w_d = nc.dram_tensor("w", [C * KW * KW, HW], F16, kind="ExternalInput")
    o_d = nc.dram_tensor("out", [C, HW], F16, kind="ExternalOutput")

    w_v = w_d[:].rearrange("(c k) m -> c k m", k=KW * KW)

    # stripe row-counts: uniform 16-row stripes. Smaller stripes measure
    # WORSE: an L-row stripe DMAs with L*192-byte descriptors, and the
    # SDMA rate collapses below ~3KB/descriptor (4-row stripe measured
    # ~140 GB/s vs ~384 GB/s at 16 rows). Startup doesn't need a small
    # first stripe either - compute is gated on the ~9 us framework
    # preamble + x head load, not on the first slab. The tail keeps the
    # per-tap load pipelining below, now at full descriptor size.
    stripe_rows = (16, 16, 16, 16, 16, 16)
    assert sum(stripe_rows) == H
    LAST = len(stripe_rows) - 1

    with TileContext(nc) as tc:
        with (
            tc.tile_pool(name="px", bufs=1) as px,
            tc.tile_pool(name="pw", bufs=10) as pw,
            tc.tile_pool(name="pg", bufs=3) as pg,
            tc.tile_pool(name="pp", bufs=2, space="PSUM") as pp,
        ):
            xa = px.tile([C, GX], F16)
            xb = px.tile([C, GX], F16)
            # x head chunk gates the first products; 21 rows cover
            # stripes 0-1 incl. halo. x rides the ACT HWDGE ring,
            # keeping the SP ring pure weights.
            nc.scalar.dma_start(
                out=xa[:, GPA : GPA + XQ[0]], in_=x_d[:, 0 : XQ[0]]
            )
            nc.gpsimd.memset(xa[:, 0:GPA], 0.0)
            nc.gpsimd.memset(xa[:, GPA + HW : GX], 0.0)
            nc.gpsimd.memset(xb[:, 0:GPB], 0.0)
            nc.gpsimd.memset(xb[:, GPB + HW : GX], 0.0)
            # shifted copy feeding the dj=+-1 taps (ScalarE: DVE is
            # loaded with the products, GPSIMD contends with DVE)
            nc.scalar.copy(
                out=xb[:, GPB : GPB + XQ[0]], in_=xa[:, GPA : GPA + XQ[0]]
            )
            ident_f = px.tile([C, C], F32)
            make_identity(nc, ident_f)
            ident = px.tile([C, C], F16)
            nc.vector.tensor_copy(out=ident[:, :], in_=ident_f[:, :])

            r0 = 0
            for si, rr in enumerate(stripe_rows):
                n = rr * W
                slabs = {}
                per_tap = {}  # gname -> taps loaded separately
                for gname, k0, di in GROUPS:
                    slab = pw.tile(
                        [C, KW, SL], F16, tag="w", name=f"w_{gname}_{si}"
                    )
                    if (si == 0 and gname == "mid") or si == LAST:
                        # startup/tail-critical: load tap-by-tap so each
                        # product (and on the last stripe each PE
                        # accumulate) can start after the smallest
                        # possible DMA footprint
                        for t in (1, 0, 2):
                            nc.sync.dma_start(
                                out=slab[:, t, 0:n],
                                in_=w_v[:, k0 + t, r0 * W : (r0 + rr) * W],
                            )
                        per_tap[gname] = True
                    else:
                        nc.sync.dma_start(
                            out=slab[:, :, 0 : rr * W],
                            in_=w_v[:, k0 : k0 + KW, r0 * W : (r0 + rr) * W],
                        )
                    slabs[gname] = slab
                if si == 0:
                    # rest of x, still ACT ring; issued before the xb
                    # copies so the ring descriptors enter back-to-back
                    nc.scalar.dma_start(
                        out=xa[:, GPA + XQ[0] : GPA + XQ[0] + XQ[1]],
                        in_=x_d[:, XQ[0] : XQ[0] + XQ[1]],
                    )
                    nc.scalar.dma_start(
                        out=xa[:, GPA + XQ[0] + XQ[1] : GPA + HW],
                        in_=x_d[:, XQ[0] + XQ[1] : HW],
                    )
                    nc.scalar.copy(
                        out=xb[:, GPB + XQ[0] : GPB + XQ[0] + XQ[1]],
                        in_=xa[:, GPA + XQ[0] : GPA + XQ[0] + XQ[1]],
                    )
                if si == 1:
                    nc.scalar.copy(
                        out=xb[:, GPB + XQ[0] + XQ[1] : GPB + HW],
                        in_=xa[:, GPA + XQ[0] + XQ[1] : GPA + HW],
                    )

                # products, in place (slab *= x window), one DVE
                # tensor_mul per tap; every AP is step-1 and
                # even-element-aligned so the DVE runs 2 fp16/cycle
                for gname, k0, di in GROUPS:
                    slab = slabs[gname]
                    row = r0 + di
                    for t in (1, 0, 2):
                        if t == 1:
                            src, off = xa, GPA + row * W
                        elif t == 0:
                            src, off = xb, GPB + row * W - 1
                        else:
                            src, off = xb, GPB + row * W + 1
                        nc.vector.tensor_mul(
                            out=slab[:, t, 0:n],
                            in0=slab[:, t, 0:n],
                            in1=src[:, off : off + n],
                        )

                # 9-way tap sum on TensorE: fp16 identity matmuls
                # accumulate the product planes into PSUM (fp32)
                acc_ps = pp.tile([C, SL], F32, tag="acc", space="PSUM")
                n_ft = (n + 511) // 512
                for j in range(n_ft):
                    f0, f1 = j * 512, min((j + 1) * 512, n)
                    for i_t, (gname, t) in enumerate(TAP_ORDER):
                        nc.tensor.matmul(
                            acc_ps[:, f0:f1],
                            ident[:, :],
                            slabs[gname][:, t, f0:f1],
                            start=(i_t == 0),
                            stop=(i_t == len(TAP_ORDER) - 1),
                        )

                # evacuate PSUM -> SBUF on ScalarE (own ports) with the
                # f32->fp16 cast fused in; store on the ACT HWDGE ring
                stg = pg.tile([C, SL], F16, tag="stg")
                nc.scalar.copy(out=stg[:, 0:n], in_=acc_ps[:, 0:n])
                nc.scalar.dma_start(
                    out=o_d[:, r0 * W : (r0 + rr) * W], in_=stg[:, 0:n]
                )
                r0 += rr

    return nc


def _split_excess_waits(nc: bass.Bass) -> None:
    """TPB engine instructions carry exactly ONE sync-wait slot; walrus
    refuses instructions with more ("Too many sync wait commands"). Tile's
    sem assignment can emit several waits on one instruction. Split the
    extras onto same-engine NOPs inserted immediately before the
    instruction — the engine sequencer executes them in order, so all
    waits are still satisfied before the instruction runs."""
    import bass_rust

    f = nc.m.functions[0]

    def make_nop(engine):
        ins = nc.engines[engine].nop().ins
        # nop() appends to the currently-open bb; detach it from there
        for bb in f.blocks:
            il = bb.instructions
            for j in range(len(il) - 1, -1, -1):
                if il[j].name == ins.name:
                    del il[j]
                    return ins
        raise AssertionError("freshly created nop not found in any block")

    for bb in f.blocks:
        il = bb.instructions
        i = 0
        while i < len(il):
            ins = il[i]
            si = ins.sync_info
            waits = list(si.on_wait) if si and si.on_wait else []
            if len(waits) > 1:
                updates = list(si.on_update) if si.on_update else []
                ins.sync_info = bass_rust.SyncInfo(
                    on_wait=[waits[-1]], on_update=updates
                )
                for k, w in enumerate(waits[:-1]):
                    nop = make_nop(ins.engine)
                    nop.sync_info = bass_rust.SyncInfo(on_wait=[w], on_update=[])
                    il.insert(i + k, nop)
                i += len(waits) - 1
            i += 1


_NC_CACHE = None


def _get_nc():
    global _NC_CACHE
    if _NC_CACHE is None:
        nc = _build()
        _split_excess_waits(nc)
        _NC_CACHE = nc
    return _NC_CACHE


_RUNNER = None


def _get_runner():
    """Jit the SPMD executable once; repeated kernel() calls reuse it.

    Mirrors concourse.bass2jax.run_bass_via_pjrt's multi-core branch but
    caches the jitted callable (run_bass_via_pjrt builds a fresh closure
    per call, forcing an XLA recompile every time)."""
    global _RUNNER
    if _RUNNER is not None:
        return _RUNNER

    import jax
    from jax.experimental.shard_map import shard_map
    from jax.sharding import Mesh, PartitionSpec

    import concourse.mybir as _mybir
    from concourse import bass2jax

    bass2jax.install_neuronx_cc_hook()
    nc = _get_nc()

    partition_name = (
        nc.partition_id_tensor.name if nc.partition_id_tensor else None
    )
    in_names, out_names, out_avals = [], [], []
    for alloc in nc.m.functions[0].allocations:
        if not isinstance(alloc, _mybir.MemoryLocationSet):
            continue
        name = alloc.memorylocations[0].name
        if alloc.kind == "ExternalInput":
            if name != partition_name:
                in_names.append(name)
        elif alloc.kind == "ExternalOutput":
            out_names.append(name)
            out_avals.append(
                jax.core.ShapedArray(
                    tuple(alloc.tensor_shape), _mybir.dt.np(alloc.dtype)
                )
            )
    n_params = len(in_names)
    n_outs = len(out_names)
    all_in_names = tuple(in_names + out_names)
    if partition_name is not None:
        all_in_names = all_in_names + (partition_name,)
    donate = tuple(range(n_params, n_params + n_outs))

    def _body(*args):
        operands = list(args)
        if partition_name is not None:
            operands.append(bass2jax.partition_id_tensor())
        outs = bass2jax._bass_exec_p.bind(
            *operands,
            out_avals=tuple(out_avals),
            in_names=all_in_names,
            out_names=tuple(out_names),
            lowering_input_output_aliases=(),
            sim_require_finite=True,
            sim_require_nnan=True,
            nc=nc,
        )
        return tuple(outs)

    devices = jax.devices()[:N_CORES]
    mesh = Mesh(np.asarray(devices), ("core",))
    sharded = jax.jit(
        shard_map(
            _body,
            mesh=mesh,
            in_specs=(PartitionSpec("core"),) * (n_params + n_outs),
            out_specs=(PartitionSpec("core"),) * n_outs,
            check_rep=False,
        ),
        donate_argnums=donate,
        keep_unused=True,
    )

    def runner(concat_inputs):
        zeros = [
            np.zeros((N_CORES * a.shape[0], *a.shape[1:]), a.dtype) for a in out_avals
        ]
        outs = sharded(*concat_inputs, *zeros)
        return [np.asarray(o) for o in outs]

    _RUNNER = (runner, in_names, out_names, out_avals)
    return _RUNNER


def _to_f16(x, conv_weights):
    """fp16 device copies of the full inputs, border weight columns
    zeroed: tap dj=-1 at w=0 and dj=+1 at w=W-1 multiply padding zeros
    in the reference, so zeroing them is exact — and it lets the device
    kernel skip the horizontal-padding memsets (its taps wrap to the
    neighbor row's edge pixel there, killed by these zero weights).
    astype() makes private copies; the caller's arrays are not touched."""
    x = np.asarray(x, dtype=np.float32)
    w = np.asarray(conv_weights, dtype=np.float32)
    assert x.shape == (N_CORES, C, H, W), x.shape
    assert w.shape == (N_CORES, C * KW * KW, H, W), w.shape
    xh = x.astype(np.float16)
    wh = w.astype(np.float16).reshape(N_CORES, C, KW * KW, H, W)
    wh[:, :, 0::KW, :, 0] = 0
    wh[:, :, KW - 1 :: KW, :, W - 1] = 0
    return xh, wh


def prep_inputs(x, conv_weights):
    """Reshape full inputs into the concatenated per-core fp16 layout."""
    xh, wh = _to_f16(x, conv_weights)
    by_name = {
        "x": np.ascontiguousarray(xh.reshape(N_CORES * C, HW)),
        "w": np.ascontiguousarray(wh.reshape(N_CORES * C * KW * KW, HW)),
    }
    _, in_names, _, _ = _get_runner()
    return [by_name[n] for n in in_names]


def execute(concat_inputs):
    runner, _, out_names, out_avals = _get_runner()
    outs = runner(concat_inputs)
    i = out_names.index("out")
    return outs[i].reshape(N_CORES, C, H, W).astype(np.float32)


def kernel(x, conv_weights):
    return execute(prep_inputs(x, conv_weights))


def run(x, conv_weights, **spmd_kwargs):
    """Legacy full-path entry via run_bass_kernel_spmd (no jit caching)."""
    xh, wh = _to_f16(x, conv_weights)
    n = xh.shape[0]
    nc = _get_nc()
    in_maps = [
        {
            "x": np.ascontiguousarray(xh[i].reshape(C, HW)),
            "w": np.ascontiguousarray(wh[i].reshape(C * KW * KW, HW)),
        }
        for i in range(n)
    ]
    br = run_bass_kernel_spmd(nc, in_maps, core_ids=list(range(n)), **spmd_kwargs)
    out = np.stack([r["out"].reshape(C, H, W).astype(np.float32) for r in br.results])
    return out, br


# revision 9
# speedup vs baseline: 1.1392x; 1.1392x over previous
"""Involution-style per-pixel depthwise 3x3 conv on 8 trn2 NeuronCores.

out[n,c,h,w] = sum_{k=0..8} w[n,c,k,h,w] * x_pad[n,c,h+k//3,w+k%3]  (pad=1)

Sharding: pure data parallel over N=8 -> one sample per core.
Per core: channels C=128 = SBUF partition dim; free dim = H*W pixels.

The kernel is HBM-bandwidth bound: with f32 tensors the mandatory
~52 MB/core at the ~358 GB/s HBM-per-NC limit puts the roofline at
~145 us (measured 142-168 us depending on paired-NC contention). The
2e-2 rel-err gate leaves ~100x of precision headroom, so the device
side runs entirely in FP16: the host converts x and w to fp16 (and
zeroes the border weight columns), the device streams ~26 MB instead
of ~52 MB, and the store is fp16 upcast to f32 on the host. fp16
roofline: ~72-85 us. Measured end-to-end rel err ~4e-4.

Design notes (carried over from the f32 kernel where still relevant):
- x lives in SBUF inside zero guard rows TWICE, at element offsets 98
  (xa) and 99 (xb). A tap (di,dj) reads a fully contiguous window;
  row overruns land in the zero guards (vertical padding); column
  wraps read the neighbor row's edge pixel and are killed by the
  border weight columns zeroed on the host (horizontal padding).
  Why two copies: DVE tensor_tensor only reaches 2x_1P mode (2 fp16
  elems/cycle) when every AP is step-1 AND 4B-aligned. Tap dj=0 reads
  at even element offsets from xa; taps dj=+-1 read at odd offsets of
  the x origin, which are even offsets in xb (x shifted by one). All
  9 products/pixel then run at 2/cycle: ~48 us of DVE, under the DMA
  roofline. xb is built from xa with chunked ScalarE copies (no extra
  HBM traffic).
- Per row-stripe, per row-group (taps sharing a row shift di), one
  DMA brings the 3-tap fp16 weight slab; one DVE tensor_mul per tap
  forms the product in place (fp16 out, exact f32 multiply inside).
- The 9-way tap sum runs on the otherwise-idle TensorE as fp16
  identity matmuls accumulating into f32 PSUM (1 PE-cycle/row).
  ScalarE evacuates PSUM->SBUF with the f32->fp16 cast fused into the
  activation copy; GPSIMD is NOT used for elementwise work (it
  contends with DVE on the shared SBUF port pair).
- The SP HWDGE ring carries ONLY the weight stream; x loads and
  output stores ride the ACT ring so their sem-waits never
  head-of-line block the weight stream. 8 slab buffers keep >2
  stripes of DMA in flight, the first stripe is small so the pipeline
  fills early, and the LAST stripe is 4 rows loaded tap-by-tap with
  the mul/matmul pipelined per tap, so only ~2 us of work remains
  after the final weight byte lands.
"""

import numpy as np

import concourse.bass as bass
import concourse.mybir as mybir
from concourse.bass_utils import run_bass_kernel_spmd
from concourse.masks import make_identity
from concourse.tile import TileContext

N_CORES = 8
C, H, W = 128, 96, 96
HW = H * W
KW = 3

R = 16                # max stripe rows (slab/psum tile sizing)
SL = R * W            # elems per stripe per partition

F16 = mybir.dt.float16
F32 = mybir.dt.float32

# row-groups: (name, first tap k0, row shift di)
GROUPS = (("mid", 3, 0), ("top", 0, -1), ("bot", 6, 1))
# tap order used for PE accumulation: mid first (its slab is DMA'd
# first, so the PE can start earliest), t=1 (dj=0, only needs xa)
# before the xb taps within each group
TAP_ORDER = (
    ("mid", 1), ("mid", 0), ("mid", 2),
    ("top", 1), ("top", 0), ("top", 2),
    ("bot", 1), ("bot", 0), ("bot", 2),
)

# guarded x layouts: xa = [98 zeros | x | 98 zeros], xb = [99 | x | 97]
# Both give 4B-aligned (even-element) window starts: xa serves dj=0
# (offset 98 + row*W, W=96 even), xb serves dj=-1 (98 + row*W) and
# dj=+1 (100 + row*W).
GPA = 98
GPB = 99
GX = GPA + HW + GPA  # 9412, also == GPB + HW + 97

# x arrives in three chunks so the xb shift-copies can chase the DMA
XQ = (21 * W, 37 * W, 38 * W)


def _build() -> bass.Bass:
    # no partition-id parameter: the kernel is SPMD-identical per core
    # and never branches on its core index
    nc = bass.Bass(enable_partition_id=False)
    x_d = nc.dram_tensor("x", [C, HW], F16, kind="ExternalInput")
    w_d = nc.dram_tensor("w", [C * KW * KW, HW], F16, kind="ExternalInput")
    o_d = nc.dram_tensor("out", [C, HW], F16, kind="ExternalOutput")

    w_v = w_d[:].rearrange("(c k) m -> c k m", k=KW * KW)

    # stripe row-counts: an L-row stripe DMAs with L*192-byte
    # descriptors, and the SDMA rate collapses below ~3KB/descriptor
    # (4-row stripe measured ~140 GB/s vs ~384 GB/s at 16 rows), so the
    # bulk runs 16-row stripes. The last stripes shrink to 8 rows to
    # bound the post-stream tail: a 16-row final stripe leaves ~5 us of
    # DVE muls + 27 matmuls + evac + store after the last weight byte
    # (measured), an 8-row one about half that. Startup doesn't need a
    # small first stripe - compute is gated on the ~9 us framework
    # preamble + x head load, not on the first slab.
    stripe_rows = (16, 16, 16, 16, 16, 8, 8)
    assert sum(stripe_rows) == H
    LAST = len(stripe_rows) - 1

    with TileContext(nc) as tc:
        with (
            tc.tile_pool(name="px", bufs=1) as px,
            tc.tile_pool(name="pw", bufs=10) as pw,
            tc.tile_pool(name="pg", bufs=3) as pg,
            tc.tile_pool(name="pp", bufs=2, space="PSUM") as pp,
        ):
            xa = px.tile([C, GX], F16)
            xb = px.tile([C, GX], F16)
            # x head chunk gates the first products; 21 rows cover
            # stripes 0-1 incl. halo. x rides the ACT HWDGE ring,
            # keeping the SP ring pure weights.
            nc.scalar.dma_start(
                out=xa[:, GPA : GPA + XQ[0]], in_=x_d[:, 0 : XQ[0]]
            )
            nc.gpsimd.memset(xa[:, 0:GPA], 0.0)
            nc.gpsimd.memset(xa[:, GPA + HW : GX], 0.0)
            nc.gpsimd.memset(xb[:, 0:GPB], 0.0)
            nc.gpsimd.memset(xb[:, GPB + HW : GX], 0.0)
            # shifted copy feeding the dj=+-1 taps (ScalarE: DVE is
            # loaded with the products, GPSIMD contends with DVE)
            nc.scalar.copy(
                out=xb[:, GPB : GPB + XQ[0]], in_=xa[:, GPA : GPA + XQ[0]]
            )
            ident_f = px.tile([C, C], F32)
            make_identity(nc, ident_f)
            ident = px.tile([C, C], F16)
            nc.vector.tensor_copy(out=ident[:, :], in_=ident_f[:, :])

            r0 = 0
            for si, rr in enumerate(stripe_rows):
                n = rr * W
                slabs = {}
                per_tap = {}  # gname -> taps loaded separately
                for gname, k0, di in GROUPS:
                    slab = pw.tile(
                        [C, KW, SL], F16, tag="w", name=f"w_{gname}_{si}"
                    )
                    if (si == 0 and gname == "mid") or si == LAST:
                        # startup/tail-critical: load tap-by-tap so each
                        # product (and on the last stripe each PE
                        # accumulate) can start after the smallest
                        # possible DMA footprint
                        for t in (1, 0, 2):
                            nc.sync.dma_start(
                                out=slab[:, t, 0:n],
                                in_=w_v[:, k0 + t, r0 * W : (r0 + rr) * W],
                            )
                        per_tap[gname] = True
                    else:
                        nc.sync.dma_start(
                            out=slab[:, :, 0 : rr * W],
                            in_=w_v[:, k0 : k0 + KW, r0 * W : (r0 + rr) * W],
                        )
                    slabs[gname] = slab
                if si == 0:
                    # rest of x, still ACT ring; issued before the xb
                    # copies so the ring descriptors enter back-to-back
                    nc.scalar.dma_start(
                        out=xa[:, GPA + XQ[0] : GPA + XQ[0] + XQ[1]],
                        in_=x_d[:, XQ[0] : XQ[0] + XQ[1]],
                    )
                    nc.scalar.dma_start(
                        out=xa[:, GPA + XQ[0] + XQ[1] : GPA + HW],
                        in_=x_d[:, XQ[0] + XQ[1] : HW],
                    )
                    nc.scalar.copy(
                        out=xb[:, GPB + XQ[0] : GPB + XQ[0] + XQ[1]],
                        in_=xa[:, GPA + XQ[0] : GPA + XQ[0] + XQ[1]],
                    )
                if si == 1:
                    nc.scalar.copy(
                        out=xb[:, GPB + XQ[0] + XQ[1] : GPB + HW],
                        in_=xa[:, GPA + XQ[0] + XQ[1] : GPA + HW],
                    )

                # products, in place (slab *= x window), one DVE
                # tensor_mul per tap; every AP is step-1 and
                # even-element-aligned so the DVE runs 2 fp16/cycle
                for gname, k0, di in GROUPS:
                    slab = slabs[gname]
                    row = r0 + di
                    for t in (1, 0, 2):
                        if t == 1:
                            src, off = xa, GPA + row * W
                        elif t == 0:
                            src, off = xb, GPB + row * W - 1
                        else:
                            src, off = xb, GPB + row * W + 1
                        nc.vector.tensor_mul(
                            out=slab[:, t, 0:n],
                            in0=slab[:, t, 0:n],
                            in1=src[:, off : off + n],
                        )

                # 9-way tap sum on TensorE: fp16 identity matmuls
                # accumulate the product planes into PSUM (fp32).
                # Whole-stripe PSUM tiles, 2 bufs: with the 8-row tail
                # stripes the WAR reuse (stripe s waits on evac s-2)
                # resolves ~10 us before the tail, so this never stalls.
                # (Rotating single-bank per-chunk tiles measured ~1.5 us
                # faster but intermittently corrupted output on HW -
                # PSUM start_tensor_calc pending-zero covers the whole
                # 2 KB bank, and the finer rotation opens a race the
                # static scheduler model doesn't see.)
                acc_ps = pp.tile([C, SL], F32, tag="acc", space="PSUM")
                n_ft = (n + 511) // 512
                for j in range(n_ft):
                    f0, f1 = j * 512, min((j + 1) * 512, n)
                    for i_t, (gname, t) in enumerate(TAP_ORDER):
                        nc.tensor.matmul(
                            acc_ps[:, f0:f1],
                            ident[:, :],
                            slabs[gname][:, t, f0:f1],
                            start=(i_t == 0),
                            stop=(i_t == len(TAP_ORDER) - 1),
                        )

                # evacuate PSUM -> SBUF on ScalarE (own ports) with the
                # f32->fp16 cast fused in; store on the ACT HWDGE ring
                stg = pg.tile([C, SL], F16, tag="stg")
                nc.scalar.copy(out=stg[:, 0:n], in_=acc_ps[:, 0:n])
                nc.scalar.dma_start(
                    out=o_d[:, r0 * W : (r0 + rr) * W], in_=stg[:, 0:n]
                )
                r0 += rr

    return nc


def _split_excess_waits(nc: bass.Bass) -> None:
    """TPB engine instructions carry exactly ONE sync-wait slot; walrus
    refuses instructions with more ("Too many sync wait commands"). Tile's
    sem assignment can emit several waits on one instruction. Split the
    extras onto same-engine NOPs inserted immediately before the
    instruction — the engine sequencer executes them in order, so all
    waits are still satisfied before the instruction runs."""
    import bass_rust

    f = nc.m.functions[0]

    def make_nop(engine):
        ins = nc.engines[engine].nop().ins
        # nop() appends to the currently-open bb; detach it from there
        for bb in f.blocks:
            il = bb.instructions
            for j in range(len(il) - 1, -1, -1):
                if il[j].name == ins.name:
                    del il[j]
                    return ins
        raise AssertionError("freshly created nop not found in any block")

    for bb in f.blocks:
        il = bb.instructions
        i = 0
        while i < len(il):
            ins = il[i]
            si = ins.sync_info
            waits = list(si.on_wait) if si and si.on_wait else []
            if len(waits) > 1:
                updates = list(si.on_update) if si.on_update else []
                ins.sync_info = bass_rust.SyncInfo(
                    on_wait=[waits[-1]], on_update=updates
                )
                for k, w in enumerate(waits[:-1]):
                    nop = make_nop(ins.engine)
                    nop.sync_info = bass_rust.SyncInfo(on_wait=[w], on_update=[])
                    il.insert(i + k, nop)
                i += len(waits) - 1
            i += 1


_NC_CACHE = None


def _get_nc():
    global _NC_CACHE
    if _NC_CACHE is None:
        nc = _build()
        _split_excess_waits(nc)
        _NC_CACHE = nc
    return _NC_CACHE


_RUNNER = None


def _get_runner():
    """Jit the SPMD executable once; repeated kernel() calls reuse it.

    Mirrors concourse.bass2jax.run_bass_via_pjrt's multi-core branch but
    caches the jitted callable (run_bass_via_pjrt builds a fresh closure
    per call, forcing an XLA recompile every time)."""
    global _RUNNER
    if _RUNNER is not None:
        return _RUNNER

    import jax
    from jax.experimental.shard_map import shard_map
    from jax.sharding import Mesh, PartitionSpec

    import concourse.mybir as _mybir
    from concourse import bass2jax

    bass2jax.install_neuronx_cc_hook()
    nc = _get_nc()

    partition_name = (
        nc.partition_id_tensor.name if nc.partition_id_tensor else None
    )
    in_names, out_names, out_avals = [], [], []
    for alloc in nc.m.functions[0].allocations:
        if not isinstance(alloc, _mybir.MemoryLocationSet):
            continue
        name = alloc.memorylocations[0].name
        if alloc.kind == "ExternalInput":
            if name != partition_name:
                in_names.append(name)
        elif alloc.kind == "ExternalOutput":
            out_names.append(name)
            out_avals.append(
                jax.core.ShapedArray(
                    tuple(alloc.tensor_shape), _mybir.dt.np(alloc.dtype)
                )
            )
    n_params = len(in_names)
    n_outs = len(out_names)
    all_in_names = tuple(in_names + out_names)
    if partition_name is not None:
        all_in_names = all_in_names + (partition_name,)
    donate = tuple(range(n_params, n_params + n_outs))

    def _body(*args):
        operands = list(args)
        if partition_name is not None:
            operands.append(bass2jax.partition_id_tensor())
        outs = bass2jax._bass_exec_p.bind(
            *operands,
            out_avals=tuple(out_avals),
            in_names=all_in_names,
            out_names=tuple(out_names),
            lowering_input_output_aliases=(),
            sim_require_finite=True,
            sim_require_nnan=True,
            nc=nc,
        )
        return tuple(outs)

    devices = jax.devices()[:N_CORES]
    mesh = Mesh(np.asarray(devices), ("core",))
    sharded = jax.jit(
        shard_map(
            _body,
            mesh=mesh,
            in_specs=(PartitionSpec("core"),) * (n_params + n_outs),
            out_specs=(PartitionSpec("core"),) * n_outs,
            check_rep=False,
        ),
        donate_argnums=donate,
        keep_unused=True,
    )

    def runner(concat_inputs):
        zeros = [
            np.zeros((N_CORES * a.shape[0], *a.shape[1:]), a.dtype) for a in out_avals
        ]
        outs = sharded(*concat_inputs, *zeros)
        return [np.asarray(o) for o in outs]

    _RUNNER = (runner, in_names, out_names, out_avals)
    return _RUNNER


def _to_f16(x, conv_weights):
    """fp16 device copies of the full inputs, border weight columns
    zeroed: tap dj=-1 at w=0 and dj=+1 at w=W-1 multiply padding zeros
    in the reference, so zeroing them is exact — and it lets the device
    kernel skip the horizontal-padding memsets (its taps wrap to the
    neighbor row's edge pixel there, killed by these zero weights).
    astype() makes private copies; the caller's arrays are not touched."""
    x = np.asarray(x, dtype=np.float32)
    w = np.asarray(conv_weights, dtype=np.float32)
    assert x.shape == (N_CORES, C, H, W), x.shape
    assert w.shape == (N_CORES, C * KW * KW, H, W), w.shape
    xh = x.astype(np.float16)
    wh = w.astype(np.float16).reshape(N_CORES, C, KW * KW, H, W)
    wh[:, :, 0::KW, :, 0] = 0
    wh[:, :, KW - 1 :: KW, :, W - 1] = 0
    return xh, wh


def prep_inputs(x, conv_weights):
    """Reshape full inputs into the concatenated per-core fp16 layout."""
    xh, wh = _to_f16(x, conv_weights)
    by_name = {
        "x": np.ascontiguousarray(xh.reshape(N_CORES * C, HW)),
        "w": np.ascontiguousarray(wh.reshape(N_CORES * C * KW * KW, HW)),
    }
    _, in_names, _, _ = _get_runner()
    return [by_name[n] for n in in_names]


def execute(concat_inputs):
    runner, _, out_names, out_avals = _get_runner()
    outs = runner(concat_inputs)
    i = out_names.index("out")
    return outs[i].reshape(N_CORES, C, H, W).astype(np.float32)


def kernel(x, conv_weights):
    return execute(prep_inputs(x, conv_weights))


def run(x, conv_weights, **spmd_kwargs):
    """Legacy full-path entry via run_bass_kernel_spmd (no jit caching)."""
    xh, wh = _to_f16(x, conv_weights)
    n = xh.shape[0]
    nc = _get_nc()
    in_maps = [
        {
            "x": np.ascontiguousarray(xh[i].reshape(C, HW)),
            "w": np.ascontiguousarray(wh[i].reshape(C * KW * KW, HW)),
        }
        for i in range(n)
    ]
    br = run_bass_kernel_spmd(nc, in_maps, core_ids=list(range(n)), **spmd_kwargs)
    out = np.stack([r["out"].reshape(C, H, W).astype(np.float32) for r in br.results])
    return out, br


# revision 11
# speedup vs baseline: 1.1527x; 1.0118x over previous
"""Involution-style per-pixel depthwise 3x3 conv on 8 trn2 NeuronCores.

out[n,c,h,w] = sum_{k=0..8} w[n,c,k,h,w] * x_pad[n,c,h+k//3,w+k%3]  (pad=1)

Sharding: pure data parallel over N=8 -> one sample per core.
Per core: channels C=128 = SBUF partition dim; free dim = H*W pixels.

The kernel is HBM-bandwidth bound: with f32 tensors the mandatory
~52 MB/core at the ~358 GB/s HBM-per-NC limit puts the roofline at
~145 us (measured 142-168 us depending on paired-NC contention). The
2e-2 rel-err gate leaves ~100x of precision headroom, so the device
side runs entirely in FP16: the host converts x and w to fp16 (and
zeroes the border weight columns), the device streams ~26 MB instead
of ~52 MB, and the store is fp16 upcast to f32 on the host. fp16
roofline: ~72-85 us. Measured end-to-end rel err ~4e-4.

Design notes (carried over from the f32 kernel where still relevant):
- x lives in SBUF inside zero guard rows TWICE, at element offsets 98
  (xa) and 99 (xb). A tap (di,dj) reads a fully contiguous window;
  row overruns land in the zero guards (vertical padding); column
  wraps read the neighbor row's edge pixel and are killed by the
  border weight columns zeroed on the host (horizontal padding).
  Why two copies: DVE tensor_tensor only reaches 2x_1P mode (2 fp16
  elems/cycle) when every AP is step-1 AND 4B-aligned. Tap dj=0 reads
  at even element offsets from xa; taps dj=+-1 read at odd offsets of
  the x origin, which are even offsets in xb (x shifted by one). All
  9 products/pixel then run at 2/cycle: ~48 us of DVE, under the DMA
  roofline. xb is built from xa with chunked ScalarE copies (no extra
  HBM traffic).
- Per row-stripe, per row-group (taps sharing a row shift di), one
  DMA brings the 3-tap fp16 weight slab; one DVE tensor_mul per tap
  forms the product in place (fp16 out, exact f32 multiply inside).
- The 9-way tap sum runs on the otherwise-idle TensorE as fp16
  identity matmuls accumulating into f32 PSUM (1 PE-cycle/row).
  ScalarE evacuates PSUM->SBUF with the f32->fp16 cast fused into the
  activation copy; GPSIMD is NOT used for elementwise work (it
  contends with DVE on the shared SBUF port pair).
- The SP HWDGE ring carries ONLY the weight stream; x loads and
  output stores ride the ACT ring so their sem-waits never
  head-of-line block the weight stream. 8 slab buffers keep >2
  stripes of DMA in flight, the first stripe is small so the pipeline
  fills early, and the LAST stripe is 4 rows loaded tap-by-tap with
  the mul/matmul pipelined per tap, so only ~2 us of work remains
  after the final weight byte lands.
"""

import numpy as np

import concourse.bass as bass
import concourse.mybir as mybir
from concourse.bass_utils import run_bass_kernel_spmd
from concourse.masks import make_identity
from concourse.tile import TileContext

N_CORES = 8
C, H, W = 128, 96, 96
HW = H * W
KW = 3

R = 16                # max stripe rows (slab/psum tile sizing)
SL = R * W            # elems per stripe per partition

F16 = mybir.dt.float16
F32 = mybir.dt.float32

# row-groups: (name, first tap k0, row shift di)
GROUPS = (("mid", 3, 0), ("top", 0, -1), ("bot", 6, 1))
# tap order used for PE accumulation: mid first (its slab is DMA'd
# first, so the PE can start earliest), t=1 (dj=0, only needs xa)
# before the xb taps within each group
TAP_ORDER = (
    ("mid", 1), ("mid", 0), ("mid", 2),
    ("top", 1), ("top", 0), ("top", 2),
    ("bot", 1), ("bot", 0), ("bot", 2),
)

# guarded x layouts: xa = [98 zeros | x | 98 zeros], xb = [99 | x | 97]
# Both give 4B-aligned (even-element) window starts: xa serves dj=0
# (offset 98 + row*W, W=96 even), xb serves dj=-1 (98 + row*W) and
# dj=+1 (100 + row*W).
GPA = 98
GPB = 99
GX = GPA + HW + GPA  # 9412, also == GPB + HW + 97

# x arrives in three chunks so the xb shift-copies can chase the DMA
XQ = (21 * W, 37 * W, 38 * W)


def _build() -> bass.Bass:
    # no partition-id parameter: the kernel is SPMD-identical per core
    # and never branches on its core index
    nc = bass.Bass(enable_partition_id=False)
    x_d = nc.dram_tensor("x", [C, HW], F16, kind="ExternalInput")
    w_d = nc.dram_tensor("w", [C * KW * KW, HW], F16, kind="ExternalInput")
    o_d = nc.dram_tensor("out", [C, HW], F16, kind="ExternalOutput")

    w_v = w_d[:].rearrange("(c k) m -> c k m", k=KW * KW)

    # stripe row-counts: an L-row stripe DMAs with L*192-byte
    # descriptors, and the SDMA rate collapses below ~3KB/descriptor
    # (4-row stripe measured ~140 GB/s vs ~384 GB/s at 16 rows), so the
    # bulk runs 16-row stripes. The last stripes shrink to 8 rows to
    # bound the post-stream tail: a 16-row final stripe leaves ~5 us of
    # DVE muls + 27 matmuls + evac + store after the last weight byte
    # (measured), an 8-row one about half that. Startup doesn't need a
    # small first stripe - compute is gated on the ~9 us framework
    # preamble + x head load, not on the first slab.
    stripe_rows = (16, 16, 16, 16, 16, 8, 8)
    assert sum(stripe_rows) == H
    LAST = len(stripe_rows) - 1

    with TileContext(nc) as tc:
        with (
            tc.tile_pool(name="px", bufs=1) as px,
            tc.tile_pool(name="pw", bufs=10) as pw,
            tc.tile_pool(name="pg", bufs=3) as pg,
            tc.tile_pool(name="pp", bufs=2, space="PSUM") as pp,
            tc.tile_pool(name="pl", bufs=1, space="PSUM") as pl,
        ):
            xa = px.tile([C, GX], F16)
            xb = px.tile([C, GX], F16)
            # x head chunk gates the first products; 21 rows cover
            # stripes 0-1 incl. halo. x rides the ACT HWDGE ring,
            # keeping the SP ring pure weights.
            nc.scalar.dma_start(
                out=xa[:, GPA : GPA + XQ[0]], in_=x_d[:, 0 : XQ[0]]
            )
            nc.gpsimd.memset(xa[:, 0:GPA], 0.0)
            nc.gpsimd.memset(xa[:, GPA + HW : GX], 0.0)
            nc.gpsimd.memset(xb[:, 0:GPB], 0.0)
            nc.gpsimd.memset(xb[:, GPB + HW : GX], 0.0)
            # shifted copy feeding the dj=+-1 taps (ScalarE: DVE is
            # loaded with the products, GPSIMD contends with DVE)
            nc.scalar.copy(
                out=xb[:, GPB : GPB + XQ[0]], in_=xa[:, GPA : GPA + XQ[0]]
            )
            ident_f = px.tile([C, C], F32)
            make_identity(nc, ident_f)
            ident = px.tile([C, C], F16)
            nc.vector.tensor_copy(out=ident[:, :], in_=ident_f[:, :])

            r0 = 0
            for si, rr in enumerate(stripe_rows):
                n = rr * W
                slabs = {}
                per_tap = {}  # gname -> taps loaded separately
                for gname, k0, di in GROUPS:
                    slab = pw.tile(
                        [C, KW, SL], F16, tag="w", name=f"w_{gname}_{si}"
                    )
                    if (si == 0 and gname == "mid") or si == LAST:
                        # startup/tail-critical: load tap-by-tap so each
                        # product (and on the last stripe each PE
                        # accumulate) can start after the smallest
                        # possible DMA footprint
                        for t in (1, 0, 2):
                            nc.sync.dma_start(
                                out=slab[:, t, 0:n],
                                in_=w_v[:, k0 + t, r0 * W : (r0 + rr) * W],
                            )
                        per_tap[gname] = True
                    else:
                        nc.sync.dma_start(
                            out=slab[:, :, 0 : rr * W],
                            in_=w_v[:, k0 : k0 + KW, r0 * W : (r0 + rr) * W],
                        )
                    slabs[gname] = slab
                if si == 0:
                    # rest of x, still ACT ring; issued before the xb
                    # copies so the ring descriptors enter back-to-back
                    nc.scalar.dma_start(
                        out=xa[:, GPA + XQ[0] : GPA + XQ[0] + XQ[1]],
                        in_=x_d[:, XQ[0] : XQ[0] + XQ[1]],
                    )
                    nc.scalar.dma_start(
                        out=xa[:, GPA + XQ[0] + XQ[1] : GPA + HW],
                        in_=x_d[:, XQ[0] + XQ[1] : HW],
                    )
                    nc.scalar.copy(
                        out=xb[:, GPB + XQ[0] : GPB + XQ[0] + XQ[1]],
                        in_=xa[:, GPA + XQ[0] : GPA + XQ[0] + XQ[1]],
                    )
                if si == 1:
                    nc.scalar.copy(
                        out=xb[:, GPB + XQ[0] + XQ[1] : GPB + HW],
                        in_=xa[:, GPA + XQ[0] + XQ[1] : GPA + HW],
                    )

                # products, in place (slab *= x window), one DVE
                # tensor_mul per tap; every AP is step-1 and
                # even-element-aligned so the DVE runs 2 fp16/cycle
                for gname, k0, di in GROUPS:
                    slab = slabs[gname]
                    row = r0 + di
                    for t in (1, 0, 2):
                        if t == 1:
                            src, off = xa, GPA + row * W
                        elif t == 0:
                            src, off = xb, GPB + row * W - 1
                        else:
                            src, off = xb, GPB + row * W + 1
                        nc.vector.tensor_mul(
                            out=slab[:, t, 0:n],
                            in0=slab[:, t, 0:n],
                            in1=src[:, off : off + n],
                        )

                # 9-way tap sum on TensorE: fp16 identity matmuls
                # accumulate the product planes into PSUM (fp32).
                # Whole-stripe PSUM tiles, 2 bufs rotating; the LAST
                # stripe gets a dedicated 2-bank tile (6+2=8 banks) so
                # its matmuls never WAR-wait on an earlier evac and can
                # pipeline tap-by-tap behind the per-tap DMAs.
                # (Rotating single-bank per-chunk tiles measured ~1.5 us
                # faster but intermittently corrupted output on HW -
                # PSUM start_tensor_calc pending-zero covers the whole
                # 2 KB bank, and the finer rotation opens a race the
                # static scheduler model doesn't see.)
                if si == LAST:
                    acc_ps = pl.tile([C, SL // 2], F32, tag="acct", space="PSUM")
                else:
                    acc_ps = pp.tile([C, SL], F32, tag="acc", space="PSUM")
                n_ft = (n + 511) // 512
                # tap-major emission: each DVE product immediately feeds
                # its chunk matmuls, so PE work spreads evenly instead
                # of bunching per chunk. Bunching left multi-us PE idle
                # gaps -> HAM re-throttled the PE to 1.2 GHz -> cold
                # matmuls (379 vs 216 ns) made the PE the hidden
                # serializer of the evac chain (measured 22 us cold).
                for i_t, (gname, t) in enumerate(TAP_ORDER):
                    for j in range(n_ft):
                        f0, f1 = j * 512, min((j + 1) * 512, n)
                        nc.tensor.matmul(
                            acc_ps[:, f0:f1],
                            ident[:, :],
                            slabs[gname][:, t, f0:f1],
                            start=(i_t == 0),
                            stop=(i_t == len(TAP_ORDER) - 1),
                        )

                # evacuate PSUM -> SBUF with the f32->fp16 cast fused
                # in: ScalarE (own ports) mid-stream; the last stripe
                # goes on DVE, which is idle by then while ScalarE may
                # still be draining earlier evac/store pushes.
                stg = pg.tile([C, SL], F16, tag="stg")
                if si == LAST:
                    nc.vector.tensor_copy(out=stg[:, 0:n], in_=acc_ps[:, 0:n])
                else:
                    nc.scalar.copy(out=stg[:, 0:n], in_=acc_ps[:, 0:n])
                nc.scalar.dma_start(
                    out=o_d[:, r0 * W : (r0 + rr) * W], in_=stg[:, 0:n]
                )
                r0 += rr

    return nc


def _split_excess_waits(nc: bass.Bass) -> None:
    """TPB engine instructions carry exactly ONE sync-wait slot; walrus
    refuses instructions with more ("Too many sync wait commands"). Tile's
    sem assignment can emit several waits on one instruction. Split the
    extras onto same-engine NOPs inserted immediately before the
    instruction — the engine sequencer executes them in order, so all
    waits are still satisfied before the instruction runs."""
    import bass_rust

    f = nc.m.functions[0]

    def make_nop(engine):
        ins = nc.engines[engine].nop().ins
        # nop() appends to the currently-open bb; detach it from there
        for bb in f.blocks:
            il = bb.instructions
            for j in range(len(il) - 1, -1, -1):
                if il[j].name == ins.name:
                    del il[j]
                    return ins
        raise AssertionError("freshly created nop not found in any block")

    for bb in f.blocks:
        il = bb.instructions
        i = 0
        while i < len(il):
            ins = il[i]
            si = ins.sync_info
            waits = list(si.on_wait) if si and si.on_wait else []
            if len(waits) > 1:
                updates = list(si.on_update) if si.on_update else []
                ins.sync_info = bass_rust.SyncInfo(
                    on_wait=[waits[-1]], on_update=updates
                )
                for k, w in enumerate(waits[:-1]):
                    nop = make_nop(ins.engine)
                    nop.sync_info = bass_rust.SyncInfo(on_wait=[w], on_update=[])
                    il.insert(i + k, nop)
                i += len(waits) - 1
            i += 1


_NC_CACHE = None


def _get_nc():
    global _NC_CACHE
    if _NC_CACHE is None:
        nc = _build()
        _split_excess_waits(nc)
        _NC_CACHE = nc
    return _NC_CACHE


_RUNNER = None


def _get_runner():
    """Jit the SPMD executable once; repeated kernel() calls reuse it.

    Mirrors concourse.bass2jax.run_bass_via_pjrt's multi-core branch but
    caches the jitted callable (run_bass_via_pjrt builds a fresh closure
    per call, forcing an XLA recompile every time)."""
    global _RUNNER
    if _RUNNER is not None:
        return _RUNNER

    import jax
    from jax.experimental.shard_map import shard_map
    from jax.sharding import Mesh, PartitionSpec

    import concourse.mybir as _mybir
    from concourse import bass2jax

    bass2jax.install_neuronx_cc_hook()
    nc = _get_nc()

    partition_name = (
        nc.partition_id_tensor.name if nc.partition_id_tensor else None
    )
    in_names, out_names, out_avals = [], [], []
    for alloc in nc.m.functions[0].allocations:
        if not isinstance(alloc, _mybir.MemoryLocationSet):
            continue
        name = alloc.memorylocations[0].name
        if alloc.kind == "ExternalInput":
            if name != partition_name:
                in_names.append(name)
        elif alloc.kind == "ExternalOutput":
            out_names.append(name)
            out_avals.append(
                jax.core.ShapedArray(
                    tuple(alloc.tensor_shape), _mybir.dt.np(alloc.dtype)
                )
            )
    n_params = len(in_names)
    n_outs = len(out_names)
    all_in_names = tuple(in_names + out_names)
    if partition_name is not None:
        all_in_names = all_in_names + (partition_name,)
    donate = tuple(range(n_params, n_params + n_outs))

    def _body(*args):
        operands = list(args)
        if partition_name is not None:
            operands.append(bass2jax.partition_id_tensor())
        outs = bass2jax._bass_exec_p.bind(
            *operands,
            out_avals=tuple(out_avals),
            in_names=all_in_names,
            out_names=tuple(out_names),
            lowering_input_output_aliases=(),
            sim_require_finite=True,
            sim_require_nnan=True,
            nc=nc,
        )
        return tuple(outs)

    devices = jax.devices()[:N_CORES]
    mesh = Mesh(np.asarray(devices), ("core",))
    sharded = jax.jit(
        shard_map(
            _body,
            mesh=mesh,
            in_specs=(PartitionSpec("core"),) * (n_params + n_outs),
            out_specs=(PartitionSpec("core"),) * n_outs,
            check_rep=False,
        ),
        donate_argnums=donate,
        keep_unused=True,
    )

    def runner(concat_inputs):
        zeros = [
            np.zeros((N_CORES * a.shape[0], *a.shape[1:]), a.dtype) for a in out_avals
        ]
        outs = sharded(*concat_inputs, *zeros)
        return [np.asarray(o) for o in outs]

    _RUNNER = (runner, in_names, out_names, out_avals)
    return _RUNNER


def _to_f16(x, conv_weights):
    """fp16 device copies of the full inputs, border weight columns
    zeroed: tap dj=-1 at w=0 and dj=+1 at w=W-1 multiply padding zeros
    in the reference, so zeroing them is exact — and it lets the device
    kernel skip the horizontal-padding memsets (its taps wrap to the
    neighbor row's edge pixel there, killed by these zero weights).
    astype() makes private copies; the caller's arrays are not touched."""
    x = np.asarray(x, dtype=np.float32)
    w = np.asarray(conv_weights, dtype=np.float32)
    assert x.shape == (N_CORES, C, H, W), x.shape
    assert w.shape == (N_CORES, C * KW * KW, H, W), w.shape
    xh = x.astype(np.float16)
    wh = w.astype(np.float16).reshape(N_CORES, C, KW * KW, H, W)
    wh[:, :, 0::KW, :, 0] = 0
    wh[:, :, KW - 1 :: KW, :, W - 1] = 0
    return xh, wh


def prep_inputs(x, conv_weights):
    """Reshape full inputs into the concatenated per-core fp16 layout."""
    xh, wh = _to_f16(x, conv_weights)
    by_name = {
        "x": np.ascontiguousarray(xh.reshape(N_CORES * C, HW)),
        "w": np.ascontiguousarray(wh.reshape(N_CORES * C * KW * KW, HW)),
    }
    _, in_names, _, _ = _get_runner()
    return [by_name[n] for n in in_names]


def execute(concat_inputs):
    runner, _, out_names, out_avals = _get_runner()
    outs = runner(concat_inputs)
    i = out_names.index("out")
    return outs[i].reshape(N_CORES, C, H, W).astype(np.float32)


def kernel(x, conv_weights):
    return execute(prep_inputs(x, conv_weights))


def run(x, conv_weights, **spmd_kwargs):
    """Legacy full-path entry via run_bass_kernel_spmd (no jit caching)."""
    xh, wh = _to_f16(x, conv_weights)
    n = xh.shape[0]
    nc = _get_nc()
    in_maps = [
        {
            "x": np.ascontiguousarray(xh[i].reshape(C, HW)),
            "w": np.ascontiguousarray(wh[i].reshape(C * KW * KW, HW)),
        }
        for i in range(n)
    ]
    br = run_bass_kernel_spmd(nc, in_maps, core_ids=list(range(n)), **spmd_kwargs)
    out = np.stack([r["out"].reshape(C, H, W).astype(np.float32) for r in br.results])
    return out, br


# revision 13
# speedup vs baseline: 1.1593x; 1.0058x over previous
"""Involution-style per-pixel depthwise 3x3 conv on 8 trn2 NeuronCores.

out[n,c,h,w] = sum_{k=0..8} w[n,c,k,h,w] * x_pad[n,c,h+k//3,w+k%3]  (pad=1)

Sharding: pure data parallel over N=8 -> one sample per core.
Per core: channels C=128 = SBUF partition dim; free dim = H*W pixels.

The kernel is HBM-bandwidth bound: with f32 tensors the mandatory
~52 MB/core at the ~358 GB/s HBM-per-NC limit puts the roofline at
~145 us (measured 142-168 us depending on paired-NC contention). The
2e-2 rel-err gate leaves ~100x of precision headroom, so the device
side runs entirely in FP16: the host converts x and w to fp16 (and
zeroes the border weight columns), the device streams ~26 MB instead
of ~52 MB, and the store is fp16 upcast to f32 on the host. fp16
roofline: ~72-85 us. Measured end-to-end rel err ~4e-4.

Design notes (carried over from the f32 kernel where still relevant):
- x lives in SBUF inside zero guard rows TWICE, at element offsets 98
  (xa) and 99 (xb). A tap (di,dj) reads a fully contiguous window;
  row overruns land in the zero guards (vertical padding); column
  wraps read the neighbor row's edge pixel and are killed by the
  border weight columns zeroed on the host (horizontal padding).
  Why two copies: DVE tensor_tensor only reaches 2x_1P mode (2 fp16
  elems/cycle) when every AP is step-1 AND 4B-aligned. Tap dj=0 reads
  at even element offsets from xa; taps dj=+-1 read at odd offsets of
  the x origin, which are even offsets in xb (x shifted by one). All
  9 products/pixel then run at 2/cycle: ~48 us of DVE, under the DMA
  roofline. xb is built from xa with chunked ScalarE copies (no extra
  HBM traffic).
- Per row-stripe, per row-group (taps sharing a row shift di), one
  DMA brings the 3-tap fp16 weight slab; one DVE tensor_mul per tap
  forms the product in place (fp16 out, exact f32 multiply inside).
  Stripes are 16 rows (3 KB DMA descriptors - smaller stripes measure
  up to 2.7x lower SDMA rate), tapering to 8 rows for the tail.
- The 9-way tap sum runs on the otherwise-idle TensorE as fp16
  identity matmuls accumulating into f32 PSUM (1 PE-cycle/row),
  emitted tap-major so PE work interleaves with the DVE products -
  chunk-major bunching left multi-us PE idle gaps that HAM-throttled
  the PE to 1.2 GHz (22 us cold, measured) and made cold matmuls the
  hidden serializer of the PSUM-WAR/evac chain. ScalarE evacuates
  PSUM->SBUF with the f32->fp16 cast fused into the activation copy;
  GPSIMD is NOT used for elementwise work (it contends with DVE on
  the shared SBUF port pair).
- The SP HWDGE ring carries ONLY the weight stream; x loads and
  output stores ride the ACT ring so their sem-waits never
  head-of-line block the weight stream. 10 slab buffers keep >3
  stripes of DMA in flight. The LAST stripe is loaded tap-by-tap into
  a dedicated 2-bank PSUM tile (no WAR on earlier evacs), with
  mul/matmul pipelined per tap and evac+store split per 512-chunk on
  DVE, so only ~3 us of work remains after the final weight byte.
"""

import numpy as np

import concourse.bass as bass
import concourse.mybir as mybir
from concourse.bass_utils import run_bass_kernel_spmd
from concourse.masks import make_identity
from concourse.tile import TileContext

N_CORES = 8
C, H, W = 128, 96, 96
HW = H * W
KW = 3

R = 16                # max stripe rows (slab/psum tile sizing)
SL = R * W            # elems per stripe per partition

F16 = mybir.dt.float16
F32 = mybir.dt.float32

# row-groups: (name, first tap k0, row shift di)
GROUPS = (("mid", 3, 0), ("top", 0, -1), ("bot", 6, 1))
# tap order used for PE accumulation: mid first (its slab is DMA'd
# first, so the PE can start earliest), t=1 (dj=0, only needs xa)
# before the xb taps within each group
TAP_ORDER = (
    ("mid", 1), ("mid", 0), ("mid", 2),
    ("top", 1), ("top", 0), ("top", 2),
    ("bot", 1), ("bot", 0), ("bot", 2),
)

# guarded x layouts: xa = [98 zeros | x | 98 zeros], xb = [99 | x | 97]
# Both give 4B-aligned (even-element) window starts: xa serves dj=0
# (offset 98 + row*W, W=96 even), xb serves dj=-1 (98 + row*W) and
# dj=+1 (100 + row*W).
GPA = 98
GPB = 99
GX = GPA + HW + GPA  # 9412, also == GPB + HW + 97

# x arrives in three chunks so the xb shift-copies can chase the DMA
XQ = (21 * W, 37 * W, 38 * W)


def _build() -> bass.Bass:
    # no partition-id parameter: the kernel is SPMD-identical per core
    # and never branches on its core index
    nc = bass.Bass(enable_partition_id=False)
    x_d = nc.dram_tensor("x", [C, HW], F16, kind="ExternalInput")
    w_d = nc.dram_tensor("w", [C * KW * KW, HW], F16, kind="ExternalInput")
    o_d = nc.dram_tensor("out", [C, HW], F16, kind="ExternalOutput")

    w_v = w_d[:].rearrange("(c k) m -> c k m", k=KW * KW)

    # stripe row-counts: an L-row stripe DMAs with L*192-byte
    # descriptors, and the SDMA rate collapses below ~3KB/descriptor
    # (4-row stripe measured ~140 GB/s vs ~384 GB/s at 16 rows), so the
    # bulk runs 16-row stripes. The last stripes shrink to 8 rows to
    # bound the post-stream tail: a 16-row final stripe leaves ~5 us of
    # DVE muls + 27 matmuls + evac + store after the last weight byte
    # (measured), an 8-row one about half that. Startup doesn't need a
    # small first stripe - compute is gated on the ~9 us framework
    # preamble + x head load, not on the first slab.
    stripe_rows = (16, 16, 16, 16, 16, 8, 8)
    assert sum(stripe_rows) == H
    LAST = len(stripe_rows) - 1

    with TileContext(nc) as tc:
        with (
            tc.tile_pool(name="px", bufs=1) as px,
            tc.tile_pool(name="pw", bufs=10) as pw,
            tc.tile_pool(name="pg", bufs=3) as pg,
            tc.tile_pool(name="pp", bufs=2, space="PSUM") as pp,
            tc.tile_pool(name="pl", bufs=1, space="PSUM") as pl,
        ):
            xa = px.tile([C, GX], F16)
            xb = px.tile([C, GX], F16)
            # x head chunk gates the first products; 21 rows cover
            # stripes 0-1 incl. halo. x rides the ACT HWDGE ring,
            # keeping the SP ring pure weights.
            nc.scalar.dma_start(
                out=xa[:, GPA : GPA + XQ[0]], in_=x_d[:, 0 : XQ[0]]
            )
            nc.gpsimd.memset(xa[:, 0:GPA], 0.0)
            nc.gpsimd.memset(xa[:, GPA + HW : GX], 0.0)
            nc.gpsimd.memset(xb[:, 0:GPB], 0.0)
            nc.gpsimd.memset(xb[:, GPB + HW : GX], 0.0)
            # shifted copy feeding the dj=+-1 taps (ScalarE: DVE is
            # loaded with the products, GPSIMD contends with DVE)
            nc.scalar.copy(
                out=xb[:, GPB : GPB + XQ[0]], in_=xa[:, GPA : GPA + XQ[0]]
            )
            ident_f = px.tile([C, C], F32)
            make_identity(nc, ident_f)
            ident = px.tile([C, C], F16)
            nc.vector.tensor_copy(out=ident[:, :], in_=ident_f[:, :])

            r0 = 0
            for si, rr in enumerate(stripe_rows):
                n = rr * W
                slabs = {}
                per_tap = {}  # gname -> taps loaded separately
                for gname, k0, di in GROUPS:
                    slab = pw.tile(
                        [C, KW, SL], F16, tag="w", name=f"w_{gname}_{si}"
                    )
                    if (si == 0 and gname == "mid") or si == LAST:
                        # startup/tail-critical: load tap-by-tap so each
                        # product (and on the last stripe each PE
                        # accumulate) can start after the smallest
                        # possible DMA footprint
                        for t in (1, 0, 2):
                            nc.sync.dma_start(
                                out=slab[:, t, 0:n],
                                in_=w_v[:, k0 + t, r0 * W : (r0 + rr) * W],
                            )
                        per_tap[gname] = True
                    else:
                        nc.sync.dma_start(
                            out=slab[:, :, 0 : rr * W],
                            in_=w_v[:, k0 : k0 + KW, r0 * W : (r0 + rr) * W],
                        )
                    slabs[gname] = slab
                if si == 0:
                    # rest of x, still ACT ring; issued before the xb
                    # copies so the ring descriptors enter back-to-back
                    nc.scalar.dma_start(
                        out=xa[:, GPA + XQ[0] : GPA + XQ[0] + XQ[1]],
                        in_=x_d[:, XQ[0] : XQ[0] + XQ[1]],
                    )
                    nc.scalar.dma_start(
                        out=xa[:, GPA + XQ[0] + XQ[1] : GPA + HW],
                        in_=x_d[:, XQ[0] + XQ[1] : HW],
                    )
                    nc.scalar.copy(
                        out=xb[:, GPB + XQ[0] : GPB + XQ[0] + XQ[1]],
                        in_=xa[:, GPA + XQ[0] : GPA + XQ[0] + XQ[1]],
                    )
                if si == 1:
                    nc.scalar.copy(
                        out=xb[:, GPB + XQ[0] + XQ[1] : GPB + HW],
                        in_=xa[:, GPA + XQ[0] + XQ[1] : GPA + HW],
                    )

                # products, in place (slab *= x window), one DVE
                # tensor_mul per tap; every AP is step-1 and
                # even-element-aligned so the DVE runs 2 fp16/cycle
                for gname, k0, di in GROUPS:
                    slab = slabs[gname]
                    row = r0 + di
                    for t in (1, 0, 2):
                        if t == 1:
                            src, off = xa, GPA + row * W
                        elif t == 0:
                            src, off = xb, GPB + row * W - 1
                        else:
                            src, off = xb, GPB + row * W + 1
                        nc.vector.tensor_mul(
                            out=slab[:, t, 0:n],
                            in0=slab[:, t, 0:n],
                            in1=src[:, off : off + n],
                        )

                # 9-way tap sum on TensorE: fp16 identity matmuls
                # accumulate the product planes into PSUM (fp32).
                # Whole-stripe PSUM tiles, 2 bufs rotating; the LAST
                # stripe gets a dedicated 2-bank tile (6+2=8 banks) so
                # its matmuls never WAR-wait on an earlier evac and can
                # pipeline tap-by-tap behind the per-tap DMAs.
                # (Rotating single-bank per-chunk tiles measured ~1.5 us
                # faster but intermittently corrupted output on HW -
                # PSUM start_tensor_calc pending-zero covers the whole
                # 2 KB bank, and the finer rotation opens a race the
                # static scheduler model doesn't see.)
                if si == LAST:
                    acc_ps = pl.tile([C, SL // 2], F32, tag="acct", space="PSUM")
                else:
                    acc_ps = pp.tile([C, SL], F32, tag="acc", space="PSUM")
                n_ft = (n + 511) // 512
                # tap-major emission: each DVE product immediately feeds
                # its chunk matmuls, so PE work spreads evenly instead
                # of bunching per chunk. Bunching left multi-us PE idle
                # gaps -> HAM re-throttled the PE to 1.2 GHz -> cold
                # matmuls (379 vs 216 ns) made the PE the hidden
                # serializer of the evac chain (measured 22 us cold).
                for i_t, (gname, t) in enumerate(TAP_ORDER):
                    for j in range(n_ft):
                        f0, f1 = j * 512, min((j + 1) * 512, n)
                        nc.tensor.matmul(
                            acc_ps[:, f0:f1],
                            ident[:, :],
                            slabs[gname][:, t, f0:f1],
                            start=(i_t == 0),
                            stop=(i_t == len(TAP_ORDER) - 1),
                        )

                # evacuate PSUM -> SBUF with the f32->fp16 cast fused
                # in: ScalarE (own ports) mid-stream; the last stripe
                # goes on DVE, which is idle by then while ScalarE may
                # still be draining earlier evac/store pushes - and is
                # split per 512-chunk with its own store so chunk 0's
                # store descriptor generation overlaps chunk 1's final
                # matmul+cast instead of serializing after it.
                stg = pg.tile([C, SL], F16, tag="stg")
                if si == LAST:
                    for j in range(n_ft):
                        f0, f1 = j * 512, min((j + 1) * 512, n)
                        nc.vector.tensor_copy(
                            out=stg[:, f0:f1], in_=acc_ps[:, f0:f1]
                        )
                        nc.scalar.dma_start(
                            out=o_d[:, r0 * W + f0 : r0 * W + f1],
                            in_=stg[:, f0:f1],
                        )
                else:
                    nc.scalar.copy(out=stg[:, 0:n], in_=acc_ps[:, 0:n])
                    nc.scalar.dma_start(
                        out=o_d[:, r0 * W : (r0 + rr) * W], in_=stg[:, 0:n]
                    )
                r0 += rr

    return nc


def _split_excess_waits(nc: bass.Bass) -> None:
    """TPB engine instructions carry exactly ONE sync-wait slot; walrus
    refuses instructions with more ("Too many sync wait commands"). Tile's
    sem assignment can emit several waits on one instruction. Split the
    extras onto same-engine NOPs inserted immediately before the
    instruction — the engine sequencer executes them in order, so all
    waits are still satisfied before the instruction runs."""
    import bass_rust

    f = nc.m.functions[0]

    def make_nop(engine):
        ins = nc.engines[engine].nop().ins
        # nop() appends to the currently-open bb; detach it from there
        for bb in f.blocks:
            il = bb.instructions
            for j in range(len(il) - 1, -1, -1):
                if il[j].name == ins.name:
                    del il[j]
                    return ins
        raise AssertionError("freshly created nop not found in any block")

    for bb in f.blocks:
        il = bb.instructions
        i = 0
        while i < len(il):
            ins = il[i]
            si = ins.sync_info
            waits = list(si.on_wait) if si and si.on_wait else []
            if len(waits) > 1:
                updates = list(si.on_update) if si.on_update else []
                ins.sync_info = bass_rust.SyncInfo(
                    on_wait=[waits[-1]], on_update=updates
                )
                for k, w in enumerate(waits[:-1]):
                    nop = make_nop(ins.engine)
                    nop.sync_info = bass_rust.SyncInfo(on_wait=[w], on_update=[])
                    il.insert(i + k, nop)
                i += len(waits) - 1
            i += 1


_NC_CACHE = None


def _get_nc():
    global _NC_CACHE
    if _NC_CACHE is None:
        nc = _build()
        _split_excess_waits(nc)
        _NC_CACHE = nc
    return _NC_CACHE


_RUNNER = None


def _get_runner():
    """Jit the SPMD executable once; repeated kernel() calls reuse it.

    Mirrors concourse.bass2jax.run_bass_via_pjrt's multi-core branch but
    caches the jitted callable (run_bass_via_pjrt builds a fresh closure
    per call, forcing an XLA recompile every time)."""
    global _RUNNER
    if _RUNNER is not None:
        return _RUNNER

    import jax
    from jax.experimental.shard_map import shard_map
    from jax.sharding import Mesh, PartitionSpec

    import concourse.mybir as _mybir
    from concourse import bass2jax

    bass2jax.install_neuronx_cc_hook()
    nc = _get_nc()

    partition_name = (
        nc.partition_id_tensor.name if nc.partition_id_tensor else None
    )
    in_names, out_names, out_avals = [], [], []
    for alloc in nc.m.functions[0].allocations:
        if not isinstance(alloc, _mybir.MemoryLocationSet):
            continue
        name = alloc.memorylocations[0].name
        if alloc.kind == "ExternalInput":
            if name != partition_name:
                in_names.append(name)
        elif alloc.kind == "ExternalOutput":
            out_names.append(name)
            out_avals.append(
                jax.core.ShapedArray(
                    tuple(alloc.tensor_shape), _mybir.dt.np(alloc.dtype)
                )
            )
    n_params = len(in_names)
    n_outs = len(out_names)
    all_in_names = tuple(in_names + out_names)
    if partition_name is not None:
        all_in_names = all_in_names + (partition_name,)
    donate = tuple(range(n_params, n_params + n_outs))

    def _body(*args):
        operands = list(args)
        if partition_name is not None:
            operands.append(bass2jax.partition_id_tensor())
        outs = bass2jax._bass_exec_p.bind(
            *operands,
            out_avals=tuple(out_avals),
            in_names=all_in_names,
            out_names=tuple(out_names),
            lowering_input_output_aliases=(),
            sim_require_finite=True,
            sim_require_nnan=True,
            nc=nc,
        )
        return tuple(outs)

    devices = jax.devices()[:N_CORES]
    mesh = Mesh(np.asarray(devices), ("core",))
    sharded = jax.jit(
        shard_map(
            _body,
            mesh=mesh,
            in_specs=(PartitionSpec("core"),) * (n_params + n_outs),
            out_specs=(PartitionSpec("core"),) * n_outs,
            check_rep=False,
        ),
        donate_argnums=donate,
        keep_unused=True,
    )

    def runner(concat_inputs):
        zeros = [
            np.zeros((N_CORES * a.shape[0], *a.shape[1:]), a.dtype) for a in out_avals
        ]
        outs = sharded(*concat_inputs, *zeros)
        return [np.asarray(o) for o in outs]

    _RUNNER = (runner, in_names, out_names, out_avals)
    return _RUNNER


def _to_f16(x, conv_weights):
    """fp16 device copies of the full inputs, border weight columns
    zeroed: tap dj=-1 at w=0 and dj=+1 at w=W-1 multiply padding zeros
    in the reference, so zeroing them is exact — and it lets the device
    kernel skip the horizontal-padding memsets (its taps wrap to the
    neighbor row's edge pixel there, killed by these zero weights).
    astype() makes private copies; the caller's arrays are not touched."""
    x = np.asarray(x, dtype=np.float32)
    w = np.asarray(conv_weights, dtype=np.float32)
    assert x.shape == (N_CORES, C, H, W), x.shape
    assert w.shape == (N_CORES, C * KW * KW, H, W), w.shape
    xh = x.astype(np.float16)
    wh = w.astype(np.float16).reshape(N_CORES, C, KW * KW, H, W)
    wh[:, :, 0::KW, :, 0] = 0
    wh[:, :, KW - 1 :: KW, :, W - 1] = 0
    return xh, wh


def prep_inputs(x, conv_weights):
    """Reshape full inputs into the concatenated per-core fp16 layout."""
    xh, wh = _to_f16(x, conv_weights)
    by_name = {
        "x": np.ascontiguousarray(xh.reshape(N_CORES * C, HW)),
        "w": np.ascontiguousarray(wh.reshape(N_CORES * C * KW * KW, HW)),
    }
    _, in_names, _, _ = _get_runner()
    return [by_name[n] for n in in_names]


def execute(concat_inputs):
    runner, _, out_names, out_avals = _get_runner()
    outs = runner(concat_inputs)
    i = out_names.index("out")
    return outs[i].reshape(N_CORES, C, H, W).astype(np.float32)


def kernel(x, conv_weights):
    return execute(prep_inputs(x, conv_weights))


def run(x, conv_weights, **spmd_kwargs):
    """Legacy full-path entry via run_bass_kernel_spmd (no jit caching)."""
    xh, wh = _to_f16(x, conv_weights)
    n = xh.shape[0]
    nc = _get_nc()
    in_maps = [
        {
            "x": np.ascontiguousarray(xh[i].reshape(C, HW)),
            "w": np.ascontiguousarray(wh[i].reshape(C * KW * KW, HW)),
        }
        for i in range(n)
    ]
    br = run_bass_kernel_spmd(nc, in_maps, core_ids=list(range(n)), **spmd_kwargs)
    out = np.stack([r["out"].reshape(C, H, W).astype(np.float32) for r in br.results])
    return out, br
